# revision 28
# baseline (speedup 1.0000x reference)
"""Multi-head causal attention (no output proj) on 8 TRN2 NeuronCores.

Problem: x[2,2048,2048] fp32, Wq/Wk/Wv[2048,2048] fp32, 16 heads of dim 128,
causal mask (fill -1e6), softmax, out = attn @ v -> [2,2048,2048] fp32.

Sharding: tensor-parallel over heads. Core c owns heads (2c, 2c+1) for both
batches: it computes Q/K/V projections for its 256 output columns and full
attention for its 4 (batch, head) instances, writing output columns
[256c : 256c+256]. No collectives.

Dataflow per core (all matmul operands fp16, PSUM accumulation fp32):
  - host supplies x^T and W slices pre-tiled to SBUF layout (fp16,
    contiguous per partition line -> large DMA packets), plus causal masks.
  - Projections: QT/KT [e, s] = W.T @ x.T per head (lhsT = W chunk, rhs = xT
    chunk); V [s, e] natural (lhsT = xT chunk, rhs = Wv chunk), stored with a
    ones column appended per head so the attn @ V matmul also produces the
    softmax denominator for free.
  - Scores, transposed: S^T[j, i] = matmul(lhsT=KT j-tile, rhs=QT i-block).
    Softmax without max-subtraction (scores ~ N(0,1); masked -> exp * 0).
    exp on ScalarE (scale=1/sqrt(128) fused), output fp16.
  - attn @ V: O[i, e+1] = sum_j matmul(lhsT=P^T tile, rhs=[V_h | ones]);
    col 128 = row sum. Normalize with per-partition reciprocal scale.
"""

import math

import numpy as np

import concourse.mybir as mybir
import concourse.tile as tile
from concourse import bacc
from concourse.bass_utils import run_bass_kernel_spmd

# ---- problem constants (hardcoded; kernel.py must be self-contained) ----
D = 2048            # model dim (contraction for projections)
S = 2048            # sequence length per batch
NB = 2              # batches
H = 2               # heads per core
E = 128             # head dim
N_CORES = 8
IBLK = 512          # i-block (query block, matmul free dim)
JT = 128            # j-tile (key tile, partition dim)
P = 128             # partitions

FP16 = mybir.dt.float16
FP32 = mybir.dt.float32


def build_program(d=D, s=S, nb=NB, h=H, e=E, iblk=IBLK):
    """Build the per-core Bass program. Returns (nc, names dict)."""
    kd = d // P                 # contraction chunks
    st = nb * s                 # total rows of x (batches flattened)
    n_sblk = st // iblk         # projection s-blocks
    sb_per_batch = s // iblk    # i-blocks per batch
    jt_per_batch = s // JT      # j-tiles per batch
    it_per_blk = iblk // P      # i-tiles per i-block
    ew = h * e                  # projection output width per core (both heads)
    vw = h * (e + 1)            # V tile width incl. ones columns

    nc = bacc.Bacc(None, target_bir_lowering=False)
    names = {}

    with tile.TileContext(nc) as tc:
        with tc.tile_pool(name="dram", bufs=1, space="DRAM") as dram:
            # host-pre-tiled layouts (contiguous per SBUF partition line, so
            # DMA moves large packets): xTt[blk, p, t, c] = x[blk*iblk+c,
            # t*128+p]; w[p, t, e] = W[t*128+p, head cols]
            xTt = dram.tile([n_sblk, P, kd, iblk], FP16, kind="ExternalInput")
            wq = dram.tile([P, kd, ew], FP16, kind="ExternalInput")
            wk = dram.tile([P, kd, ew], FP16, kind="ExternalInput")
            wv = dram.tile([P, kd, ew], FP16, kind="ExternalInput")
            msk = dram.tile([P, 3 * iblk], FP16, kind="ExternalInput")
            out = dram.tile([st, ew], FP32, kind="ExternalOutput")
            names.update(xT=xTt.name, wq=wq.name, wk=wk.name, wv=wv.name,
                         msk=msk.name, out=out.name)

            with (
                tc.tile_pool(name="wpool", bufs=1) as wpool,
                tc.tile_pool(name="xpool", bufs=3) as xpool,
                tc.tile_pool(name="qkv", bufs=1) as qkv,
                tc.tile_pool(name="ppool", bufs=18) as ppool,
                tc.tile_pool(name="opool", bufs=6) as opool,
                tc.tile_pool(name="psA", bufs=2, space="PSUM") as psA,
                tc.tile_pool(name="psB", bufs=2, space="PSUM") as psB,
            ):
                # ---- startup loads, earliest-needed first ----
                # Inputs are host-pre-tiled (contiguous per partition line ->
                # large DMA packets). Block-0 x parts and wq parts interleave
                # so the first Q matmuls unblock after ~0.75 MiB.
                def load_xtb(blk, parts=2):
                    t = xpool.tile([P, kd, iblk], FP16, tag="xT",
                                   name=f"xtb{blk}")
                    step = kd // parts
                    for q in range(parts):
                        t0, t1 = q * step, (q + 1) * step
                        nc.sync.dma_start(out=t[:, t0:t1, :],
                                          in_=xTt[blk, :, t0:t1, :])
                    return t

                def load_w(name, wd, parts=1):
                    t = wpool.tile([P, kd, ew], FP16, tag=f"w{name}",
                                   name=f"w{name}")
                    step = kd // parts
                    for q in range(parts):
                        t0, t1 = q * step, (q + 1) * step
                        nc.sync.dma_start(out=t[:, t0:t1, :],
                                          in_=wd[:, t0:t1, :])
                    return t

                # Startup stream: the single HWDGE queue drains in emission
                # order, so interleave chunk-wise in the order the block-0
                # projection chains consume: (xtb0, wq) pairs first, with wk
                # joining from the halfway point (K chains start right after
                # Q) and wv at the tail (V chains run last).
                w_sb = {}
                xtb0 = xpool.tile([P, kd, iblk], FP16, tag="xT", name="xtb0")
                w_sb["q"] = wpool.tile([P, kd, ew], FP16, tag="wq", name="wq_sb")
                w_sb["k"] = wpool.tile([P, kd, ew], FP16, tag="wk", name="wk_sb")
                w_sb["v"] = wpool.tile([P, kd, ew], FP16, tag="wv", name="wv_sb")
                nstp = max(1, kd // 8)
                nparts = kd // nstp
                for q in range(nparts):
                    t0, t1 = q * nstp, (q + 1) * nstp
                    nc.sync.dma_start(out=xtb0[:, t0:t1, :],
                                      in_=xTt[0, :, t0:t1, :])
                    nc.sync.dma_start(out=w_sb["q"][:, t0:t1, :],
                                      in_=wq[:, t0:t1, :])
                    if q >= nparts // 2:
                        k0 = (q - nparts // 2) * 2 * nstp
                        k1 = min(kd, k0 + 2 * nstp)
                        nc.sync.dma_start(out=w_sb["k"][:, k0:k1, :],
                                          in_=wk[:, k0:k1, :])
                for q in range(2):
                    t0, t1 = q * (kd // 2), (q + 1) * (kd // 2)
                    nc.sync.dma_start(out=w_sb["v"][:, t0:t1, :],
                                      in_=wv[:, t0:t1, :])
                mask_sb = wpool.tile([P, 3 * iblk], FP16, tag="mask")
                nc.sync.dma_start(out=mask_sb, in_=msk[:])

                # ---- persistent QT/KT/V in SBUF (fp16) ----
                # qT/kT: per head, [e, st] with batches side by side.
                qT = [qkv.tile([P, st], FP16, tag=f"qT{i}", name=f"qT{i}")
                      for i in range(h)]
                kT = [qkv.tile([P, st], FP16, tag=f"kT{i}", name=f"kT{i}")
                      for i in range(h)]
                # V: per (batch, j-tile): [128 j, h*(e+1)] with ones columns.
                vt = [qkv.tile([P, vw], FP16, tag=f"v{i}", name=f"v{i}")
                      for i in range(nb * jt_per_batch)]

                # ---- projections, per s-block, as per-chain thunks ----
                # (each thunk is ~2-3.5us of dense PE work; interleaving them
                # between attention score-groups hides ScalarE exp latency
                # without ever stalling the in-order PE stream)
                def proj_thunks(blk):
                    s0 = blk * iblk
                    box = {}

                    def qk(name, hh):
                        def f():
                            if "x" not in box:
                                box["x"] = (xtb0 if blk == 0
                                            else load_xtb(blk))
                            xtb = box["x"]
                            dst = {"q": qT, "k": kT}[name][hh]
                            ps = psA.tile([P, iblk], FP32, tag="psA",
                                          name="ps")
                            for t in range(kd):
                                nc.tensor.matmul(
                                    ps[:],
                                    w_sb[name][:, t, hh * e:(hh + 1) * e],
                                    xtb[:, t, :],
                                    start=(t == 0),
                                    stop=(t == kd - 1),
                                )
                            nc.vector.tensor_copy(dst[:, s0:s0 + iblk], ps[:])
                        return f

                    def vproj(it):
                        def f():
                            xtb = box["x"]
                            ps = psB.tile([P, ew], FP32, tag="psB", name="ps")
                            for t in range(kd):
                                nc.tensor.matmul(
                                    ps[:],
                                    xtb[:, t, it * P:(it + 1) * P],
                                    w_sb["v"][:, t, :],
                                    start=(t == 0),
                                    stop=(t == kd - 1),
                                )
                            v_dst = vt[(s0 + it * P) // JT]
                            for hh in range(h):
                                nc.vector.tensor_copy(
                                    v_dst[:, hh * (e + 1):hh * (e + 1) + e],
                                    ps[:, hh * e:(hh + 1) * e],
                                )
                                nc.vector.memset(
                                    v_dst[:, hh * (e + 1) + e:
                                          hh * (e + 1) + e + 1],
                                    1.0,
                                )
                        return f

                    return ([qk(n, hh) for n in ("q", "k")
                             for hh in range(h)]
                            + [vproj(it) for it in range(it_per_blk)])

                # ---- attention, software-pipelined ----
                # For each (batch, head, i-block) step: scores+exp for step
                # k+1 are emitted before the attn@V matmuls of step k, so the
                # PE never stalls waiting on ScalarE's exp.
                inv_sqrt_e = 1.0 / math.sqrt(e)

                def scores_thunks(b, hh, ib, p_tiles):
                    """Per-group thunks for one i-block's scores+exp+mask.

                    Full j-tiles go three-per-PSUM-tile (3 banks; one wide
                    exp covers all three). The 4 narrowed diagonal j-tiles
                    pack into ONE 3-bank tile: bank0 = d0[512], bank1 =
                    d1[384] + d3[128] (one accumulation group, disjoint
                    writes), bank2 = d2[256]; a single exp + one host-built
                    mask handle the whole diagonal. Each thunk appends
                    per-jt (p_tile, eff) entries to p_tiles; the PV lhsT
                    slice for i-tile t is p_tile[:, t*128+eff :][:128].
                    """
                    i0 = b * s + ib * iblk
                    n_full = it_per_blk * ib

                    def score_mm(sp, base, jt, c0, start=True, stop=True):
                        nc.tensor.matmul(
                            sp[:, base:base + (iblk - c0)],
                            kT[hh][:, b * s + jt * JT:b * s + (jt + 1) * JT],
                            qT[hh][:, i0 + c0:i0 + iblk],
                            start=start,
                            stop=stop,
                        )

                    def full_group(g0):
                        def f():
                            gn = min(3, n_full - g0)
                            sp = psA.tile([P, 3 * iblk], FP32, tag="psA",
                                          name="sp")
                            pt = ppool.tile([P, 3 * iblk], FP16, tag="p",
                                            name="pt")
                            for k in range(gn):
                                score_mm(sp, k * iblk, g0 + k, 0)
                                p_tiles.append((pt, k * iblk))
                            nc.scalar.activation(
                                pt[:, 0:gn * iblk], sp[:, 0:gn * iblk],
                                mybir.ActivationFunctionType.Exp,
                                scale=inv_sqrt_e,
                            )
                        return f

                    def diag_quad():
                        sp = psA.tile([P, 3 * iblk], FP32, tag="psA",
                                      name="sp")
                        pt = ppool.tile([P, 3 * iblk], FP16, tag="p",
                                        name="pt")
                        q0 = n_full
                        score_mm(sp, 0, q0 + 0, 0)                # d0 [0:512]
                        score_mm(sp, iblk, q0 + 1, P, stop=False)
                        score_mm(sp, iblk + 384, q0 + 3, 3 * P,
                                 start=False)                     # d3
                        score_mm(sp, 2 * iblk, q0 + 2, 2 * P)     # d2
                        p_tiles.append((pt, 0))                   # d0: eff 0
                        p_tiles.append((pt, iblk - P))            # d1: eff 384
                        p_tiles.append((pt, 2 * iblk - 2 * P))    # d2: eff 768
                        p_tiles.append((pt, iblk + 384 - 3 * P))  # d3: eff 512
                        tw = 2 * iblk + 256
                        nc.scalar.activation(
                            pt[:, 0:tw], sp[:, 0:tw],
                            mybir.ActivationFunctionType.Exp,
                            scale=inv_sqrt_e,
                        )
                        nc.vector.tensor_mul(
                            pt[:, 0:tw], pt[:, 0:tw], mask_sb[:, 0:tw]
                        )

                    return ([full_group(g0) for g0 in range(0, n_full, 3)]
                            + [diag_quad])

                def pv_thunks(b, hh, ib, p_tiles):
                    i0 = b * s + ib * iblk
                    jbase = b * jt_per_batch

                    def one(it):
                        def f():
                            op = psB.tile([P, e + 1], FP32, tag="psB",
                                          name="op")
                            last = it_per_blk * ib + it
                            for jt in range(last + 1):
                                pt, eff = p_tiles[jt]
                                lo = it * P + eff
                                nc.tensor.matmul(
                                    op[:],
                                    pt[:, lo:lo + P],
                                    vt[jbase + jt][:, hh * (e + 1):
                                                   (hh + 1) * (e + 1)],
                                    start=(jt == 0),
                                    stop=(jt == last),
                                )
                            rec = opool.tile([P, 1], FP32, tag="rec",
                                             name="rec")
                            nc.vector.reciprocal(rec[:], op[:, e:e + 1])
                            ot = opool.tile([P, e], FP32, tag="o", name="ot")
                            nc.vector.tensor_scalar_mul(ot[:], op[:, 0:e],
                                                        rec[:])
                            r0 = i0 + it * P
                            nc.sync.dma_start(
                                out=out[r0:r0 + P, hh * e:(hh + 1) * e],
                                in_=ot[:],
                            )
                        return f

                    return [one(it) for it in range(it_per_blk)]

                # ---- interleaved emission ----
                # Attention step (b, hh, ib) becomes ready once projection
                # s-block b*sb_per_batch+ib is emitted. Its score-group
                # thunks are queued immediately, its attn@V thunks one step
                # later (so scores of the next step always precede attn@V of
                # the previous -> no exp-latency stall). Between every two
                # attention thunks one projection-chain thunk is emitted:
                # dense PE work that hides ScalarE's exp under the PE-bound
                # projection phase.
                from collections import deque

                attn_q = deque()
                pending_pv = None
                step_list = sorted(
                    [(b, hh, ib) for b in range(nb) for hh in range(h)
                     for ib in range(sb_per_batch)],
                    key=lambda st: (st[0] * sb_per_batch + st[2], st[1]),
                )
                si = 0

                def queue_ready(blk_done):
                    nonlocal si, pending_pv
                    while (si < len(step_list)
                           and step_list[si][0] * sb_per_batch
                           + step_list[si][2] <= blk_done):
                        st = step_list[si]
                        si += 1
                        shared = []
                        sc = [("sc", t) for t in scores_thunks(*st, shared)]
                        pv = ([("pv", t) for t in pending_pv]
                              if pending_pv is not None else [])
                        # zip score-groups with the previous step's attn@V
                        # thunks: each attn@V group is ~1.5us of PE work that
                        # covers the exp latency of the preceding scores
                        merged = []
                        for k in range(max(len(sc), len(pv))):
                            if k < len(sc):
                                merged.append(sc[k])
                            if k < len(pv):
                                merged.append(pv[k])
                        attn_q.extend(merged)
                        pending_pv = pv_thunks(*st, shared)

                for blk in range(n_sblk):
                    for th in proj_thunks(blk):
                        th()
                        if attn_q:
                            attn_q.popleft()[1]()
                        # drain a backlog faster with an extra attn@V thunk
                        # (uses psB only -> no PSUM contention with scores)
                        if len(attn_q) > 20 and attn_q[0][0] == "pv":
                            attn_q.popleft()[1]()
                    queue_ready(blk)
                while attn_q:
                    attn_q.popleft()[1]()
                if pending_pv is not None:
                    for th in pending_pv:
                        th()

    nc.compile()
    return nc, names


def host_tile_x(x_flat, iblk, p=P):
    """[st, d] -> [n_sblk, p, kd, iblk] with layout x[blk*iblk+c, t*p+pp]."""
    st, d = x_flat.shape
    return np.ascontiguousarray(
        x_flat.reshape(st // iblk, iblk, d // p, p).transpose(0, 3, 2, 1)
        .astype(np.float16)
    )


def host_tile_w(w_cols, p=P):
    """[d, ew] -> [p, kd, ew] with layout W[t*p+pp, e]."""
    d, ew = w_cols.shape
    return np.ascontiguousarray(
        w_cols.reshape(d // p, p, ew).transpose(1, 0, 2).astype(np.float16)
    )


def host_mask(iblk, p=P):
    """Causal mask [p, 3*iblk] for the packed diagonal quad layout:
    cols [0:512]=d0, [512:896]=d1(384), [896:1024]=d3(128), [1024:1280]=d2
    (256). Every narrowed diagonal tile reduces to the base pattern
    diag[pp, c] = (pp <= c)."""
    diag = (np.arange(p)[:, None] <= np.arange(iblk)[None, :])
    m = np.zeros((p, 3 * iblk), dtype=np.float16)
    m[:, 0:iblk] = diag
    m[:, iblk:iblk + 384] = diag[:, 0:384]
    m[:, iblk + 384:iblk + 512] = diag[:, 0:128]
    m[:, 2 * iblk:2 * iblk + 256] = diag[:, 0:256]
    return m


def _host_prep(x, Wq, Wk, Wv):
    """Shard + cast inputs on host. Returns list of 8 in_maps."""
    st = x.shape[0] * x.shape[1]
    xTt = host_tile_x(x.reshape(st, D), IBLK)
    msk = host_mask(IBLK)
    in_maps = []
    for c in range(N_CORES):
        cols = slice(2 * c * E, 2 * (c + 1) * E)
        in_maps.append({
            "xT": xTt,
            "wq": host_tile_w(Wq[:, cols]),
            "wk": host_tile_w(Wk[:, cols]),
            "wv": host_tile_w(Wv[:, cols]),
            "msk": msk,
        })
    return in_maps


_CACHE = {}


def _get_program():
    if "nc" not in _CACHE:
        nc, names = build_program()
        _CACHE["nc"] = nc
        _CACHE["names"] = names
    return _CACHE["nc"], _CACHE["names"]


def kernel(x, Wq, Wk, Wv, _trace=False, _tmpdir=None):
    nc, names = _get_program()
    raw_maps = _host_prep(np.asarray(x), np.asarray(Wq), np.asarray(Wk),
                          np.asarray(Wv))
    in_maps = [{names[k]: v for k, v in m.items()} for m in raw_maps]
    res = run_bass_kernel_spmd(
        nc, in_maps, core_ids=list(range(N_CORES)),
        trace=_trace, tmpdir=_tmpdir,
    )
    b, s, d = x.shape
    out = np.empty((b, s, d), dtype=np.float32)
    for c in range(N_CORES):
        core_out = res.results[c][names["out"]]  # [4096, 256]
        out[:, :, 2 * c * E:2 * (c + 1) * E] = core_out.reshape(b, s, 2 * E)
    if _trace:
        _CACHE["last_results"] = res
    return out
